# revision 15
# baseline (speedup 1.0000x reference)
"""Trainium2 Bass kernel for the synaptic-depression RNN (nn_Actor).

Reference computation (per batch b, t = 0..T-1):
    h_{t+1} = sigmoid((y_t * h_t) @ W_hh + x_t @ W_ih)
    y_{t+1} = y_t + 0.1*(-(y_t-1)(1-h_{t+1}) - (y_t-0.25) h_{t+1})
            = 0.9*y_t + 0.1 - 0.075*h_{t+1}          (exact algebraic identity)
    outputs: mean = hs @ mean_w.T + mean_b, std = clip(hs @ std_w.T + std_b, -20, 2),
             h_last, all_hs, y_last

Sharding: data-parallel over B=64 -> 8 cores x 8 batch. Weights replicated.
Per-core layout is H-major ("transposed"): state tiles are [128, 32] where
column j*8+b holds h[b, j*128+p] for partition p. The recurrent matmul is
z.T[ho,b] = sum_hi W_hh[hi,ho] g.T[hi,b] with W_hh chunks as the stationary
operand (bf16 -> fast weight load) and g [128,8] as the moving operand.
U = x @ W_ih is computed by matmuls that *initialize* the PSUM accumulators
(start=True) for up to 25 future timesteps; the 16 recurrent matmuls per step
then accumulate on top (start=False). Head matmuls run from the bf16 hs ring
one step behind to fill the PE pipeline while ACT/DVE run the serial chain.
"""

import sys

sys.path.insert(0, "/opt/trn_rl_repo")

from contextlib import ExitStack

import numpy as np

import concourse.bacc as bacc
import concourse.bass as bass
import concourse.tile as tile
from concourse import mybir
from concourse.bass_utils import run_bass_kernel_spmd

F32 = mybir.dt.float32
BF16 = mybir.dt.bfloat16
AF = mybir.ActivationFunctionType
OP = mybir.AluOpType

B = 8           # local batch per core
TC = 50         # timesteps per psum chunk
LOG_SIG_MIN, LOG_SIG_MAX = -20.0, 2.0


def build(T=1000):
    """Build the per-core Bass graph. T must be a multiple of 50."""
    assert T % TC == 0
    niter = T // TC

    nc = bacc.Bacc(None, target_bir_lowering=False, debug=False)

    xT = nc.declare_dram_parameter("xT", [128, B, T], F32, isOutput=False)
    h0 = nc.declare_dram_parameter("h0", [128, 32], F32, isOutput=False)
    y0 = nc.declare_dram_parameter("y0", [128, 32], F32, isOutput=False)
    whh = nc.declare_dram_parameter("whh", [512, 512], F32, isOutput=False)
    wih = nc.declare_dram_parameter("wih", [128, 512], F32, isOutput=False)
    whd = nc.declare_dram_parameter("whd", [512, 32], F32, isOutput=False)
    # bhd[:, 0] = bias, bhd[:, 1] = upper clip, bhd[:, 2] = lower clip
    bhd = nc.declare_dram_parameter("bhd", [32, 3], F32, isOutput=False)

    hs_out = nc.declare_dram_parameter("hs", [128, T, 32], BF16, isOutput=True)
    ms_out = nc.declare_dram_parameter("ms", [32, B, T], F32, isOutput=True)
    hlast_out = nc.declare_dram_parameter("hlast", [128, 32], BF16, isOutput=True)
    ylast_out = nc.declare_dram_parameter("ylast", [128, 32], F32, isOutput=True)

    with tile.TileContext(nc) as tc, ExitStack() as ctx:
        singles = ctx.enter_context(tc.tile_pool(name="singles", bufs=1))

        whb = [
            singles.tile([128, 512], BF16, name=f"whb{k}", tag=f"whb{k}")
            for k in range(4)
        ]
        wib = singles.tile([128, 512], BF16)
        whdb = singles.tile([128, 4, 32], BF16)
        bhd_sb = singles.tile([32, 3], F32)
        xtb = singles.tile([128, B, T], BF16)
        hs_buf = singles.tile([128, T, 32], BF16)
        ms_buf = singles.tile([32, B, T], F32)
        y_sb = singles.tile([128, 32], F32)
        # Per-group state tiles: dependency tracking is tile-granular, so each
        # batch group gets its OWN g/b/h tiles -- group A's matmuls must not
        # serialize behind group B's chain writes.
        g_gr = [
            singles.tile([128, 4, 4], BF16, name=f"g{go}", tag=f"g{go}")
            for go in range(2)
        ]
        b_gr = [
            singles.tile([128, 4, 4], F32, name=f"bg{go}", tag=f"bg{go}")
            for go in range(2)
        ]
        # ping-pong (by t parity) per group
        h_gr = [
            [
                singles.tile([128, 4, 4], BF16, name=f"h{go}{i}", tag=f"h{go}{i}")
                for i in range(2)
            ]
            for go in range(2)
        ]

        with tc.tile_pool(name="wstage", bufs=2) as wstage, \
             tc.tile_pool(name="xstage", bufs=1) as xstage:
            for k in range(4):
                st = wstage.tile([128, 512], F32, tag="wst")
                nc.sync.dma_start(out=st, in_=whh[k * 128:(k + 1) * 128, :])
                nc.vector.tensor_copy(out=whb[k], in_=st)
            st = wstage.tile([128, 512], F32, tag="wst")
            nc.sync.dma_start(out=st, in_=wih[:, :])
            nc.vector.tensor_copy(out=wib, in_=st)

            st = wstage.tile([128, 4, 32], F32, tag="wsd")
            nc.sync.dma_start(out=st, in_=whd[:, :].rearrange("(k p) a -> p k a", p=128))
            nc.vector.tensor_copy(out=whdb, in_=st)
            nc.sync.dma_start(out=bhd_sb, in_=bhd[:, :])

            xs = xstage.tile([128, B, T], F32)
            nc.sync.dma_start(out=xs, in_=xT[:, :, :])
            nc.vector.tensor_copy(out=xtb, in_=xs)

            h0s = wstage.tile([128, 32], F32, tag="wsi")
            y0s = wstage.tile([128, 32], F32, tag="wsi")
            nc.sync.dma_start(out=h0s, in_=h0[:, :])
            nc.sync.dma_start(out=y0s, in_=y0[:, :])
            h0v = h0s.rearrange("p (j b) -> p j b", j=4)
            y0v = y0s.rearrange("p (j b) -> p j b", j=4)
            for go in range(2):
                gsl = slice(go * 4, (go + 1) * 4)
                nc.vector.tensor_mul(g_gr[go], y0v[:, :, gsl], h0v[:, :, gsl])
                nc.vector.tensor_scalar(
                    b_gr[go], y0v[:, :, gsl], 0.9, 0.1, OP.mult, OP.add
                )
            nc.vector.tensor_copy(out=y_sb, in_=y0s)

        psum = ctx.enter_context(tc.tile_pool(name="psum", bufs=2, space="PSUM"))
        psumh = ctx.enter_context(tc.tile_pool(name="psumh", bufs=2, space="PSUM"))
        fpool = ctx.enter_context(tc.tile_pool(name="fpool", bufs=2))

        # Two batch groups (b 0-3 | b 4-7) run half a step out of phase so
        # one group's sigmoid->y->g serial chain hides under the other
        # group's matmuls. Each group owns 2 PSUM banks (2 j-chunks per bank).
        GB = 4  # batch per group
        yv = y_sb.rearrange("p (j b) -> p j b", j=4)

        with tc.For_i(0, niter, hint_engines=(mybir.EngineType.PE,)) as iv:
            t0 = iv * TC
            pcs = [
                psum.tile([128, 2, 2, 256], F32, name=f"pc{go}", tag=f"pc{go}", bufs=1)
                for go in range(2)
            ]
            pcvs = [
                pc[:, :, :, 0:GB * TC].rearrange("p a c (b t) -> p a c b t", b=GB)
                for pc in pcs
            ]

            # U matmuls for both groups; one start=True per bank clears the
            # whole bank's stale has_written bits
            for go in range(2):
                for j in range(4):
                    for b in range(GB):
                        nc.tensor.matmul(
                            pcs[go][:, j // 2, j % 2, b * TC:(b + 1) * TC],
                            lhsT=wib[:, j * 128:(j + 1) * 128],
                            rhs=xtb[:, go * GB + b, bass.ds(t0, TC)],
                            start=(j % 2 == 0 and b == 0),
                            stop=False,
                            skip_group_check=True,
                        )

            for t in range(TC):
                tt = t0 + t
                for go in range(2):
                    gsl = slice(go * GB, (go + 1) * GB)
                    h_cur = h_gr[go][t % 2]
                    # 16 recurrent matmuls for this group
                    for j in range(4):
                        for k in range(4):
                            nc.tensor.matmul(
                                pcvs[go][:, j // 2, j % 2, :, t],
                                lhsT=whb[k][:, j * 128:(j + 1) * 128],
                                rhs=g_gr[go][:, k, :],
                                start=False,
                                stop=(k == 3),
                                skip_group_check=True,
                            )
                    nc.scalar.activation(
                        out=h_cur,
                        in_=pcvs[go][:, :, :, :, t].rearrange("p a c b -> p (a c) b"),
                        func=AF.Sigmoid,
                    )
                    # DVE chain: y = (h * -0.075) + b ; g = y * h
                    nc.vector.scalar_tensor_tensor(
                        yv[:, :, gsl], h_cur, -0.075, b_gr[go],
                        OP.mult, OP.add,
                    )
                    nc.vector.tensor_mul(g_gr[go], yv[:, :, gsl], h_cur)
                    # off-path on GpSimd: b = 0.9*y + 0.1 ; hs ring writeback
                    nc.gpsimd.tensor_scalar(
                        b_gr[go], yv[:, :, gsl], 0.9, 0.1, OP.mult, OP.add
                    )
                    nc.gpsimd.tensor_copy(
                        out=hs_buf[:, bass.ds(tt, 1), :].rearrange(
                            "p o (j b) -> p (o j) b", j=4
                        )[:, :, gsl],
                        in_=h_cur,
                    )

        # bulk mean/std heads from the bf16 hs ring: for each 50-step chunk,
        # psum[32, (t,b)] = sum_k whd[k].T @ hs[:, chunk, k-block]; then add
        # bias + clip via transposed-view DVE reads into ms_buf (b-major)
        for c in range(T // TC):
            ph = psumh.tile([32, TC * B], F32, name="ph", tag="ph")
            for k in range(4):
                nc.tensor.matmul(
                    ph,
                    lhsT=whdb[:, k, :],
                    rhs=hs_buf[:, c * TC:(c + 1) * TC, k * 8:(k + 1) * 8],
                    start=(k == 0),
                    stop=(k == 3),
                )
            phv = ph.rearrange("a (t b) -> a t b", b=B)
            # transposed view: [32, b, t]
            phT = bass.AP(
                tensor=phv.tensor, offset=phv.offset,
                ap=[phv.ap[0], phv.ap[2], phv.ap[1]],
            )
            tmp = fpool.tile([32, B, TC], F32)
            nc.vector.tensor_scalar(
                tmp, phT, bhd_sb[:, 0:1], bhd_sb[:, 1:2], OP.add, OP.min
            )
            nc.vector.tensor_scalar(
                ms_buf[:, :, c * TC:(c + 1) * TC], tmp, bhd_sb[:, 2:3], None, OP.max
            )

        half = T // 2
        nc.sync.dma_start(out=hs_out[:, 0:half, :], in_=hs_buf[:, 0:half, :])
        nc.sync.dma_start(out=hs_out[:, half:T, :], in_=hs_buf[:, half:T, :])
        nc.sync.dma_start(out=ms_out[:, :, :], in_=ms_buf)
        nc.sync.dma_start(out=hlast_out[:, :], in_=hs_buf[:, T - 1, :])
        nc.sync.dma_start(out=ylast_out[:, :], in_=y_sb)

    nc.finalize()
    return nc


def make_in_maps(x, hn, y_depression, W_hh, W_ih, mean_w, mean_b, std_w, std_b):
    Bfull, T, INP = x.shape
    ncores = Bfull // B
    whd = np.ascontiguousarray(np.concatenate([mean_w, std_w], 0).T).astype(np.float32)
    FMAX = 3.0e38
    bias = np.concatenate([mean_b, std_b]).astype(np.float32)
    upper = np.array([FMAX] * 16 + [2.0] * 16, np.float32)
    lower = np.array([-FMAX] * 16 + [-20.0] * 16, np.float32)
    bhd = np.ascontiguousarray(np.stack([bias, upper, lower], axis=1))
    whh = np.ascontiguousarray(W_hh, dtype=np.float32)
    wih = np.ascontiguousarray(W_ih, dtype=np.float32)

    def tr_state(v):  # [8, 512] -> [128, 32] with col j*8+b = v[b, j*128+p]
        return np.ascontiguousarray(
            v.reshape(B, 4, 128).transpose(2, 1, 0).reshape(128, 32)
        ).astype(np.float32)

    in_maps = []
    for c in range(ncores):
        sl = slice(c * B, (c + 1) * B)
        in_maps.append({
            "xT": np.ascontiguousarray(x[sl].transpose(2, 0, 1)).astype(np.float32),
            "h0": tr_state(np.asarray(hn)[0, sl]),
            "y0": tr_state(np.asarray(y_depression)[0, sl]),
            "whh": whh,
            "wih": wih,
            "whd": whd,
            "bhd": bhd,
        })
    return in_maps


def assemble(results, T):
    ncores = len(results)
    Bfull = ncores * B
    mean = np.empty((Bfull, T, 16), np.float32)
    std = np.empty((Bfull, T, 16), np.float32)
    all_hs = np.empty((Bfull, T, 512), np.float32)
    h_last = np.empty((1, Bfull, 512), np.float32)
    y_last = np.empty((1, Bfull, 512), np.float32)
    for c, r in enumerate(results):
        sl = slice(c * B, (c + 1) * B)
        hs = np.asarray(r["hs"]).astype(np.float32)        # [128, T, 32]
        all_hs[sl] = hs.reshape(128, T, 4, B).transpose(3, 1, 2, 0).reshape(B, T, 512)
        ms = np.asarray(r["ms"]).astype(np.float32)        # [32, B, T]
        mean[sl] = ms[:16].transpose(1, 2, 0)
        std[sl] = ms[16:].transpose(1, 2, 0)
        hl = np.asarray(r["hlast"]).astype(np.float32)     # [128, 32]
        h_last[0, sl] = hl.reshape(128, 4, B).transpose(2, 1, 0).reshape(B, 512)
        yl = np.asarray(r["ylast"]).astype(np.float32)
        y_last[0, sl] = yl.reshape(128, 4, B).transpose(2, 1, 0).reshape(B, 512)
    return mean, std, h_last, all_hs, y_last


_NC_CACHE = {}


def run(inputs, T=1000, trace=False, **kw):
    if T not in _NC_CACHE:
        _NC_CACHE[T] = build(T)
    nc = _NC_CACHE[T]
    in_maps = make_in_maps(**inputs)
    res = run_bass_kernel_spmd(
        nc, in_maps, core_ids=list(range(len(in_maps))), trace=trace, **kw
    )
    return assemble(res.results, T), res


def kernel(**inputs):
    out, _ = run(inputs, T=1000)
    return out


# revision 16
# speedup vs baseline: 1.1864x; 1.1864x over previous
"""Trainium2 Bass kernel for the synaptic-depression RNN (nn_Actor).

Reference computation (per batch b, t = 0..T-1):
    h_{t+1} = sigmoid((y_t * h_t) @ W_hh + x_t @ W_ih)
    y_{t+1} = y_t + 0.1*(-(y_t-1)(1-h_{t+1}) - (y_t-0.25) h_{t+1})
            = 0.9*y_t + 0.1 - 0.075*h_{t+1}          (exact algebraic identity)
    outputs: mean = hs @ mean_w.T + mean_b, std = clip(hs @ std_w.T + std_b, -20, 2),
             h_last, all_hs, y_last

Sharding: data-parallel over B=64 -> 8 cores x 8 batch. Weights replicated.
Per-core layout is H-major ("transposed"): state tiles are [128, 32] where
column j*8+b holds h[b, j*128+p] for partition p. The recurrent matmul is
z.T[ho,b] = sum_hi W_hh[hi,ho] g.T[hi,b] with W_hh chunks as the stationary
operand (bf16 -> fast weight load) and g [128,8] as the moving operand.
U = x @ W_ih is computed by matmuls that *initialize* the PSUM accumulators
(start=True) for up to 25 future timesteps; the 16 recurrent matmuls per step
then accumulate on top (start=False). Head matmuls run from the bf16 hs ring
one step behind to fill the PE pipeline while ACT/DVE run the serial chain.
"""

import sys

sys.path.insert(0, "/opt/trn_rl_repo")

from contextlib import ExitStack

import numpy as np

import concourse.bacc as bacc
import concourse.bass as bass
import concourse.tile as tile
from concourse import mybir
from concourse.bass_utils import run_bass_kernel_spmd

F32 = mybir.dt.float32
BF16 = mybir.dt.bfloat16
FP8 = mybir.dt.float8e4
AF = mybir.ActivationFunctionType
OP = mybir.AluOpType

B = 8           # local batch per core
TC = 50         # timesteps per psum chunk
LOG_SIG_MIN, LOG_SIG_MAX = -20.0, 2.0


def build(T=1000):
    """Build the per-core Bass graph. T must be a multiple of 50."""
    assert T % TC == 0
    niter = T // TC

    nc = bacc.Bacc(None, target_bir_lowering=False, debug=False)

    xT = nc.declare_dram_parameter("xT", [128, B, T], F32, isOutput=False)
    h0 = nc.declare_dram_parameter("h0", [128, 32], F32, isOutput=False)
    y0 = nc.declare_dram_parameter("y0", [128, 32], F32, isOutput=False)
    whh = nc.declare_dram_parameter("whh", [512, 512], F32, isOutput=False)
    wih = nc.declare_dram_parameter("wih", [128, 512], F32, isOutput=False)
    whd = nc.declare_dram_parameter("whd", [512, 32], F32, isOutput=False)
    # bhd[:, 0] = bias, bhd[:, 1] = upper clip, bhd[:, 2] = lower clip
    bhd = nc.declare_dram_parameter("bhd", [32, 3], F32, isOutput=False)

    hs_out = nc.declare_dram_parameter("hs", [128, T, 32], BF16, isOutput=True)
    ms_out = nc.declare_dram_parameter("ms", [32, B, T], F32, isOutput=True)
    hlast_out = nc.declare_dram_parameter("hlast", [128, 32], BF16, isOutput=True)
    ylast_out = nc.declare_dram_parameter("ylast", [128, 32], F32, isOutput=True)

    with tile.TileContext(nc) as tc, ExitStack() as ctx:
        singles = ctx.enter_context(tc.tile_pool(name="singles", bufs=1))

        whb = [
            singles.tile([128, 512], FP8, name=f"whb{k}", tag=f"whb{k}")
            for k in range(4)
        ]
        wib = singles.tile([128, 512], BF16)
        whdb = singles.tile([128, 4, 32], BF16)
        bhd_sb = singles.tile([32, 3], F32)
        xtb = singles.tile([128, B, T], BF16)
        hs_buf = singles.tile([128, T, 32], BF16)
        ms_buf = singles.tile([32, B, T], F32)
        y_sb = singles.tile([128, 32], F32)
        # Per-group state tiles: dependency tracking is tile-granular, so each
        # batch group gets its OWN g/b/h tiles -- group A's matmuls must not
        # serialize behind group B's chain writes.
        g_gr = [
            singles.tile([128, 4, 4], FP8, name=f"g{go}", tag=f"g{go}")
            for go in range(2)
        ]
        b_gr = [
            singles.tile([128, 4, 4], F32, name=f"bg{go}", tag=f"bg{go}")
            for go in range(2)
        ]
        # ping-pong (by t parity) per group
        h_gr = [
            [
                singles.tile([128, 4, 4], BF16, name=f"h{go}{i}", tag=f"h{go}{i}")
                for i in range(2)
            ]
            for go in range(2)
        ]

        with tc.tile_pool(name="wstage", bufs=2) as wstage, \
             tc.tile_pool(name="xstage", bufs=1) as xstage:
            for k in range(4):
                st = wstage.tile([128, 512], F32, tag="wst")
                nc.sync.dma_start(out=st, in_=whh[k * 128:(k + 1) * 128, :])
                nc.vector.tensor_copy(out=whb[k], in_=st)
            st = wstage.tile([128, 512], F32, tag="wst")
            nc.sync.dma_start(out=st, in_=wih[:, :])
            nc.vector.tensor_copy(out=wib, in_=st)

            st = wstage.tile([128, 4, 32], F32, tag="wsd")
            nc.sync.dma_start(out=st, in_=whd[:, :].rearrange("(k p) a -> p k a", p=128))
            nc.vector.tensor_copy(out=whdb, in_=st)
            nc.sync.dma_start(out=bhd_sb, in_=bhd[:, :])

            xs = xstage.tile([128, B, T], F32)
            nc.sync.dma_start(out=xs, in_=xT[:, :, :])
            nc.vector.tensor_copy(out=xtb, in_=xs)

            h0s = wstage.tile([128, 32], F32, tag="wsi")
            y0s = wstage.tile([128, 32], F32, tag="wsi")
            nc.sync.dma_start(out=h0s, in_=h0[:, :])
            nc.sync.dma_start(out=y0s, in_=y0[:, :])
            h0v = h0s.rearrange("p (j b) -> p j b", j=4)
            y0v = y0s.rearrange("p (j b) -> p j b", j=4)
            for go in range(2):
                gsl = slice(go * 4, (go + 1) * 4)
                nc.vector.tensor_mul(g_gr[go], y0v[:, :, gsl], h0v[:, :, gsl])
                nc.vector.tensor_scalar(
                    b_gr[go], y0v[:, :, gsl], 0.9, 0.1, OP.mult, OP.add
                )
            nc.vector.tensor_copy(out=y_sb, in_=y0s)

        psum = ctx.enter_context(tc.tile_pool(name="psum", bufs=2, space="PSUM"))
        psumh = ctx.enter_context(tc.tile_pool(name="psumh", bufs=2, space="PSUM"))
        fpool = ctx.enter_context(tc.tile_pool(name="fpool", bufs=2))

        # Two batch groups (b 0-3 | b 4-7) run half a step out of phase so
        # one group's sigmoid->y->g serial chain hides under the other
        # group's matmuls. Each group owns 2 PSUM banks (2 j-chunks per bank).
        GB = 4  # batch per group
        yv = y_sb.rearrange("p (j b) -> p j b", j=4)

        with tc.For_i(0, niter, hint_engines=(mybir.EngineType.PE,)) as iv:
            t0 = iv * TC
            pcs = [
                psum.tile([128, 2, 2, 256], F32, name=f"pc{go}", tag=f"pc{go}", bufs=1)
                for go in range(2)
            ]
            pcvs = [
                pc[:, :, :, 0:GB * TC].rearrange("p a c (b t) -> p a c b t", b=GB)
                for pc in pcs
            ]

            # U matmuls for both groups; one start=True per bank clears the
            # whole bank's stale has_written bits
            for go in range(2):
                for j in range(4):
                    for b in range(GB):
                        nc.tensor.matmul(
                            pcs[go][:, j // 2, j % 2, b * TC:(b + 1) * TC],
                            lhsT=wib[:, j * 128:(j + 1) * 128],
                            rhs=xtb[:, go * GB + b, bass.ds(t0, TC)],
                            start=(j % 2 == 0 and b == 0),
                            stop=False,
                            skip_group_check=True,
                        )

            for t in range(TC):
                tt = t0 + t
                for go in range(2):
                    gsl = slice(go * GB, (go + 1) * GB)
                    h_cur = h_gr[go][t % 2]
                    # 16 recurrent matmuls for this group
                    for j in range(4):
                        for k in range(4):
                            nc.tensor.matmul(
                                pcvs[go][:, j // 2, j % 2, :, t],
                                lhsT=whb[k][:, j * 128:(j + 1) * 128],
                                rhs=g_gr[go][:, k, :],
                                start=False,
                                stop=(k == 3),
                                skip_group_check=True,
                            )
                    nc.scalar.activation(
                        out=h_cur,
                        in_=pcvs[go][:, :, :, :, t].rearrange("p a c b -> p (a c) b"),
                        func=AF.Sigmoid,
                    )
                    # DVE chain: y = (h * -0.075) + b ; g = y * h
                    nc.vector.scalar_tensor_tensor(
                        yv[:, :, gsl], h_cur, -0.075, b_gr[go],
                        OP.mult, OP.add,
                    )
                    nc.vector.tensor_mul(g_gr[go], yv[:, :, gsl], h_cur)
                    # off-path on GpSimd: b = 0.9*y + 0.1 ; hs ring writeback
                    nc.gpsimd.tensor_scalar(
                        b_gr[go], yv[:, :, gsl], 0.9, 0.1, OP.mult, OP.add
                    )
                    nc.gpsimd.tensor_copy(
                        out=hs_buf[:, bass.ds(tt, 1), :].rearrange(
                            "p o (j b) -> p (o j) b", j=4
                        )[:, :, gsl],
                        in_=h_cur,
                    )

        # bulk mean/std heads from the bf16 hs ring: for each 50-step chunk,
        # psum[32, (t,b)] = sum_k whd[k].T @ hs[:, chunk, k-block]; then add
        # bias + clip via transposed-view DVE reads into ms_buf (b-major)
        for c in range(T // TC):
            ph = psumh.tile([32, TC * B], F32, name="ph", tag="ph")
            for k in range(4):
                nc.tensor.matmul(
                    ph,
                    lhsT=whdb[:, k, :],
                    rhs=hs_buf[:, c * TC:(c + 1) * TC, k * 8:(k + 1) * 8],
                    start=(k == 0),
                    stop=(k == 3),
                )
            phv = ph.rearrange("a (t b) -> a t b", b=B)
            # transposed view: [32, b, t]
            phT = bass.AP(
                tensor=phv.tensor, offset=phv.offset,
                ap=[phv.ap[0], phv.ap[2], phv.ap[1]],
            )
            tmp = fpool.tile([32, B, TC], F32)
            nc.vector.tensor_scalar(
                tmp, phT, bhd_sb[:, 0:1], bhd_sb[:, 1:2], OP.add, OP.min
            )
            nc.vector.tensor_scalar(
                ms_buf[:, :, c * TC:(c + 1) * TC], tmp, bhd_sb[:, 2:3], None, OP.max
            )

        half = T // 2
        nc.sync.dma_start(out=hs_out[:, 0:half, :], in_=hs_buf[:, 0:half, :])
        nc.sync.dma_start(out=hs_out[:, half:T, :], in_=hs_buf[:, half:T, :])
        nc.sync.dma_start(out=ms_out[:, :, :], in_=ms_buf)
        nc.sync.dma_start(out=hlast_out[:, :], in_=hs_buf[:, T - 1, :])
        nc.sync.dma_start(out=ylast_out[:, :], in_=y_sb)

    nc.finalize()
    return nc


def make_in_maps(x, hn, y_depression, W_hh, W_ih, mean_w, mean_b, std_w, std_b):
    Bfull, T, INP = x.shape
    ncores = Bfull // B
    whd = np.ascontiguousarray(np.concatenate([mean_w, std_w], 0).T).astype(np.float32)
    FMAX = 3.0e38
    bias = np.concatenate([mean_b, std_b]).astype(np.float32)
    upper = np.array([FMAX] * 16 + [2.0] * 16, np.float32)
    lower = np.array([-FMAX] * 16 + [-20.0] * 16, np.float32)
    bhd = np.ascontiguousarray(np.stack([bias, upper, lower], axis=1))
    whh = np.ascontiguousarray(W_hh, dtype=np.float32)
    wih = np.ascontiguousarray(W_ih, dtype=np.float32)

    def tr_state(v):  # [8, 512] -> [128, 32] with col j*8+b = v[b, j*128+p]
        return np.ascontiguousarray(
            v.reshape(B, 4, 128).transpose(2, 1, 0).reshape(128, 32)
        ).astype(np.float32)

    in_maps = []
    for c in range(ncores):
        sl = slice(c * B, (c + 1) * B)
        in_maps.append({
            "xT": np.ascontiguousarray(x[sl].transpose(2, 0, 1)).astype(np.float32),
            "h0": tr_state(np.asarray(hn)[0, sl]),
            "y0": tr_state(np.asarray(y_depression)[0, sl]),
            "whh": whh,
            "wih": wih,
            "whd": whd,
            "bhd": bhd,
        })
    return in_maps


def assemble(results, T):
    ncores = len(results)
    Bfull = ncores * B
    mean = np.empty((Bfull, T, 16), np.float32)
    std = np.empty((Bfull, T, 16), np.float32)
    all_hs = np.empty((Bfull, T, 512), np.float32)
    h_last = np.empty((1, Bfull, 512), np.float32)
    y_last = np.empty((1, Bfull, 512), np.float32)
    for c, r in enumerate(results):
        sl = slice(c * B, (c + 1) * B)
        hs = np.asarray(r["hs"]).astype(np.float32)        # [128, T, 32]
        all_hs[sl] = hs.reshape(128, T, 4, B).transpose(3, 1, 2, 0).reshape(B, T, 512)
        ms = np.asarray(r["ms"]).astype(np.float32)        # [32, B, T]
        mean[sl] = ms[:16].transpose(1, 2, 0)
        std[sl] = ms[16:].transpose(1, 2, 0)
        hl = np.asarray(r["hlast"]).astype(np.float32)     # [128, 32]
        h_last[0, sl] = hl.reshape(128, 4, B).transpose(2, 1, 0).reshape(B, 512)
        yl = np.asarray(r["ylast"]).astype(np.float32)
        y_last[0, sl] = yl.reshape(128, 4, B).transpose(2, 1, 0).reshape(B, 512)
    return mean, std, h_last, all_hs, y_last


_NC_CACHE = {}


def run(inputs, T=1000, trace=False, **kw):
    if T not in _NC_CACHE:
        _NC_CACHE[T] = build(T)
    nc = _NC_CACHE[T]
    in_maps = make_in_maps(**inputs)
    res = run_bass_kernel_spmd(
        nc, in_maps, core_ids=list(range(len(in_maps))), trace=trace, **kw
    )
    return assemble(res.results, T), res


def kernel(**inputs):
    out, _ = run(inputs, T=1000)
    return out
